# revision 1
# baseline (speedup 1.0000x reference)
"""Multi-head attention (B=2, S=2048, D=1024, H=16, Dh=64, causal) on 8 TRN2 cores.

Sharding: (batch, head-group) across 8 cores -> core c handles batch c//4 and
heads [4*(c%4), 4*(c%4)+4). Wq/Wk/Wv column-sharded by head group.

Per-core kernel (all matmuls in float32r = full-accuracy fast PE mode):
  - inputs: xT [D,S] (host-transposed x), W slices [D,256]
  - v computed first in natural [S, dh] layout (starts as soon as the first
    xT chunk lands), augmented with a DMA'd ones column
  - qT/kT = W.T @ x.T computed directly in [head*dh, S] layout
  - scoresT tiles [sk=128, sq<=512] = kT.T @ qT per head; causal tiles
    fully above the diagonal are skipped, diagonal-band tiles are trimmed to
    the columns that can be nonzero and masked by precomputed 0/1 masks
  - pT = exp(scoresT/8) (no max subtraction needed; scores ~ N(0,1))
  - ctxT_aug [65, sq] += v_aug.T @ pT accumulated over sk chunks; row 64 is
    the softmax normalizer (ones-column trick)
  - normalize via DVE reciprocal + gpsimd partition_broadcast + DVE multiply
  - output octxT [4, 64, S]; host transposes back to [S, 256]
"""
import sys

if "/opt/trn_rl_repo" not in sys.path:
    sys.path.insert(0, "/opt/trn_rl_repo")

import numpy as np

import concourse.bacc as bacc
import concourse.mybir as mybir
import concourse.tile as tile
from concourse.bass_utils import run_bass_kernel_spmd

F32 = mybir.dt.float32
F32R = mybir.dt.float32r

P = 128          # partitions
S = 2048         # sequence length
D = 1024         # model dim
C = 256          # W columns per core (4 heads x 64)
DH = 64          # head dim
NH = 4           # heads per core
SQT = 512        # sq tile (matmul free dim)
NSQ = S // SQT   # 4
NSK = S // P     # 16
ND = D // P      # 8
N_CORES = 8

_NC_CACHE = {}


def build_nc(loop_n=1):
    key = ("nc", loop_n)
    if key in _NC_CACHE:
        return _NC_CACHE[key]
    nc = bacc.Bacc("TRN2")
    xT = nc.dram_tensor("xT", [D, S], F32R, kind="ExternalInput")
    wq = nc.dram_tensor("wq", [D, C], F32R, kind="ExternalInput")
    wk = nc.dram_tensor("wk", [D, C], F32R, kind="ExternalInput")
    wv = nc.dram_tensor("wv", [D, C], F32R, kind="ExternalInput")
    masks = nc.dram_tensor("masks", [P, 4, SQT], F32R, kind="ExternalInput")
    ones4 = nc.dram_tensor("ones4", [P, NSK * NH], F32R, kind="ExternalInput")
    octxT = nc.dram_tensor("octxT", [NH, DH, S], F32, kind="ExternalOutput")

    import contextlib
    with tile.TileContext(nc) as tc:
        with (tc.For_i(0, loop_n, 1) if loop_n > 1 else contextlib.nullcontext()), \
             tc.tile_pool(name="const", bufs=1) as cp, \
             tc.tile_pool(name="work", bufs=2) as wkp, \
             tc.tile_pool(name="ps", bufs=2, space="PSUM") as ps:
            # ---- persistent SBUF residents ----
            xt = [cp.tile([P, S], F32R, tag=f"xt{k}", name=f"xt{k}") for k in range(ND)]
            wq_sb = cp.tile([P, ND, C], F32R, tag="wq", name="wq_sb")
            wk_sb = cp.tile([P, ND, C], F32R, tag="wk", name="wk_sb")
            wv_sb = cp.tile([P, ND, C], F32R, tag="wv", name="wv_sb")
            mask_sb = cp.tile([P, 4, SQT], F32R, tag="mask", name="mask_sb")
            qT_sb = [cp.tile([P, S], F32R, tag=f"qT{i}", name=f"qT{i}") for i in range(2)]
            kT_sb = [cp.tile([P, S], F32R, tag=f"kT{i}", name=f"kT{i}") for i in range(2)]
            va = cp.tile([P, NSK, NH, DH + 1], F32R, tag="va", name="va")

            # ---- input DMAs (order = prefetch priority) ----
            # xt arrives in column quarters so chunk-c work starts early
            wv3 = wv.rearrange("(ko p) c -> ko p c", p=P)
            nc.sync.dma_start(wv_sb[:, 0], wv3[0])
            for k in range(ND):
                nc.sync.dma_start(xt[k][:, 0:SQT], xT[k * P:(k + 1) * P, 0:SQT])
                if k + 1 < ND:
                    nc.sync.dma_start(wv_sb[:, k + 1], wv3[k + 1])
            nc.sync.dma_start(wq_sb[:], wq.rearrange("(ko p) c -> p ko c", p=P))
            nc.sync.dma_start(wk_sb[:], wk.rearrange("(ko p) c -> p ko c", p=P))
            nc.scalar.dma_start(mask_sb[:], masks[:])
            nc.scalar.dma_start(
                va[:, :, :, DH],
                ones4.rearrange("p (j h) -> p j h", j=NSK))
            for q in range(1, NSQ):
                for k in range(ND):
                    nc.sync.dma_start(xt[k][:, q * SQT:(q + 1) * SQT],
                                      xT[k * P:(k + 1) * P, q * SQT:(q + 1) * SQT])

            def emit_proj(c):
                sq = slice(c * SQT, (c + 1) * SQT)
                for j in range(4 * c, 4 * c + 4):
                    psv = ps.tile([P, C], F32, tag="B", bufs=2, name="psv")
                    for k in range(ND):
                        nc.tensor.matmul(psv[:],
                                         xt[k][:, j * P:(j + 1) * P],
                                         wv_sb[:, k],
                                         start=(k == 0), stop=(k == ND - 1))
                    nc.vector.tensor_copy(
                        va[:, j, :, 0:DH],
                        psv[:].rearrange("p (h d) -> p h d", h=NH))
                for hp in range(2):
                    psq = ps.tile([P, SQT], F32, tag="B", bufs=2, name="psq")
                    for k in range(ND):
                        nc.tensor.matmul(psq[:],
                                         wq_sb[:, k, hp * P:(hp + 1) * P],
                                         xt[k][:, sq],
                                         start=(k == 0), stop=(k == ND - 1))
                    nc.vector.tensor_copy(qT_sb[hp][:, sq], psq[:])
                    psk = ps.tile([P, SQT], F32, tag="B", bufs=2, name="psk")
                    for k in range(ND):
                        nc.tensor.matmul(psk[:],
                                         wk_sb[:, k, hp * P:(hp + 1) * P],
                                         xt[k][:, sq],
                                         start=(k == 0), stop=(k == ND - 1))
                    nc.vector.tensor_copy(kT_sb[hp][:, sq], psk[:])

            def emit_attn(c):
                sq = slice(c * SQT, (c + 1) * SQT)
                jmax = 4 * c + 4
                for hp in range(2):
                    # head pair h0 = 2*hp, h1 = 2*hp+1 processed together so
                    # their K=64 QK matmuls sit adjacent (PE row-group overlap)
                    pscs = [ps.tile([DH + 1, SQT], F32, tag="A", bufs=2,
                                    name=f"psc{i}") for i in range(2)]
                    # untrimmed sk chunks (j < 4c) in pairs: two QK outputs in
                    # one 2-bank psum tile, ONE exp op over 1024 columns
                    for jp in range(2 * c):
                        j0 = 2 * jp
                        for i in range(2):
                            off = DH * i
                            pss = ps.tile([P, 2, SQT], F32, tag="S", bufs=2,
                                          name=f"pss{i}")
                            for u in range(2):
                                nc.tensor.matmul(pss[:, u],
                                                 kT_sb[hp][off:off + DH,
                                                           (j0 + u) * P:(j0 + u + 1) * P],
                                                 qT_sb[hp][off:off + DH, sq],
                                                 start=True, stop=True)
                            pt = wkp.tile([P, 2, SQT], F32R, tag="pT", bufs=8,
                                          name=f"pt{i}")
                            nc.scalar.activation(pt[:], pss[:],
                                                 mybir.ActivationFunctionType.Exp,
                                                 scale=0.125)
                            for u in range(2):
                                nc.tensor.matmul(pscs[i][:],
                                                 va[:, j0 + u, 2 * hp + i, :],
                                                 pt[:, u],
                                                 start=(j0 + u == 0), stop=False)
                    # diagonal band: trimmed singles with masking
                    for j in range(4 * c, jmax):
                        t = j - 4 * c
                        # keep matmul free dim >= 256: f32r runs 4x slower
                        # below 256 columns, so a 128-wide trim is a net loss
                        lo = min(P * t, SQT - 2 * P)
                        w = SQT - lo
                        sqw = slice(c * SQT + lo, (c + 1) * SQT)
                        psss = []
                        for i in range(2):
                            off = DH * i
                            pss = ps.tile([P, 2, SQT], F32, tag="S", bufs=2,
                                          name=f"pss{i}")
                            nc.tensor.matmul(pss[:, 0, 0:w],
                                             kT_sb[hp][off:off + DH, j * P:(j + 1) * P],
                                             qT_sb[hp][off:off + DH, sqw],
                                             start=True, stop=True)
                            psss.append(pss)
                        for i in range(2):
                            pt = wkp.tile([P, 2, SQT], F32R, tag="pT", bufs=8,
                                          name=f"pt{i}")
                            nc.scalar.activation(pt[:, 0, lo:SQT], psss[i][:, 0, 0:w],
                                                 mybir.ActivationFunctionType.Exp,
                                                 scale=0.125)
                            hi = min(P * t + P, SQT)
                            nc.vector.tensor_mul(pt[:, 0, lo:hi],
                                                 pt[:, 0, lo:hi],
                                                 mask_sb[:, t, lo:hi])
                            nc.tensor.matmul(pscs[i][:, lo:SQT],
                                             va[:, j, 2 * hp + i, :],
                                             pt[:, 0, lo:SQT],
                                             start=(j == 0), stop=(j == jmax - 1))
                    for i in range(2):
                        h = 2 * hp + i
                        recip = wkp.tile([1, SQT], F32, tag="recip", bufs=4,
                                         name="recip")
                        nc.vector.reciprocal(recip[:], pscs[i][DH:DH + 1, :])
                        bc = wkp.tile([DH, SQT], F32, tag="bc", bufs=4, name="bc")
                        nc.gpsimd.partition_broadcast(bc[:], recip[:])
                        ctx_sb = wkp.tile([DH, SQT], F32, tag="ctx", bufs=4,
                                          name="ctx_sb")
                        nc.vector.tensor_mul(ctx_sb[:], pscs[i][0:DH, :], bc[:])
                        nc.scalar.dma_start(octxT[h, :, sq], ctx_sb[:])

            for c in range(NSQ):
                emit_proj(c)
                emit_attn(c)
    nc.compile()
    _NC_CACHE[key] = nc
    return nc


def _masks_np():
    # mask_t[p, f] = 1.0 if (128*t + p) <= f else 0  (allowed = key pos <= query pos)
    p = np.arange(P)[:, None, None]
    t = np.arange(4)[None, :, None]
    f = np.arange(SQT)[None, None, :]
    return np.ascontiguousarray(((P * t + p) <= f).astype(np.float32))


def make_in_maps(x, Wq, Wk, Wv):
    x = np.asarray(x, dtype=np.float32)
    Wq = np.asarray(Wq, dtype=np.float32)
    Wk = np.asarray(Wk, dtype=np.float32)
    Wv = np.asarray(Wv, dtype=np.float32)
    masks = _masks_np()
    ones4 = np.ones((P, NSK * NH), np.float32)
    in_maps = []
    for core in range(N_CORES):
        b, g = divmod(core, 4)
        cols = slice(C * g, C * (g + 1))
        in_maps.append({
            "xT": np.ascontiguousarray(x[b].T),
            "wq": np.ascontiguousarray(Wq[:, cols]),
            "wk": np.ascontiguousarray(Wk[:, cols]),
            "wv": np.ascontiguousarray(Wv[:, cols]),
            "masks": masks,
            "ones4": ones4,
        })
    return in_maps


def assemble_out(results):
    out = np.empty((2, S, D), np.float32)
    for core in range(N_CORES):
        b, g = divmod(core, 4)
        octxT = results[core]["octxT"]            # [4, 64, S]
        out[b, :, C * g:C * (g + 1)] = octxT.transpose(2, 0, 1).reshape(S, C)
    return out


def kernel(x, Wq, Wk, Wv):
    nc = build_nc()
    in_maps = make_in_maps(x, Wq, Wk, Wv)
    res = run_bass_kernel_spmd(nc, in_maps, core_ids=list(range(N_CORES)))
    return assemble_out(res.results)



# revision 10
# speedup vs baseline: 1.1454x; 1.1454x over previous
"""Multi-head attention (B=2, S=2048, D=1024, H=16, Dh=64, causal) on 8 TRN2 cores.

Sharding: (batch, head-group) across 8 cores -> core c handles batch c//4 and
heads [4*(c%4), 4*(c%4)+4). Wq/Wk/Wv column-sharded by head group.

Per-core kernel, all matmul operands bf16 (f32 PSUM accumulation):
  - inputs: xtq [4, 128, 8, 512] bf16 (host-transposed x, quarter-major),
    W slices [128, 8, 256] bf16, triangle mask [128, 2, 128] bf16
  - warmup matmuls on zeroed scratch cover the input-DMA latency and the PE
    p-state ramp (full clock needs ~3us of continuous PE busy)
  - projections contract D in 8 chunks of 128; qT/kT produced directly in
    [head*dh, S] bf16 layout; v in [sk, 65] layout with a memset ones column
    (row 64 of the PV accumulator is the softmax normalizer)
  - scoresT tiles [sk=128, sq<=512] = kT.T @ qT per head; causal tiles fully
    above the diagonal are skipped; diagonal-band tiles are trimmed to the
    128*t boundary and their first 128 columns masked by a 0/1 triangle
  - pT = exp(scoresT/8) on the Act engine (bf16 out, no max subtraction:
    scores ~ N(0,1), bf16 range is ample); both heads of a band tile share
    one exp instruction to amortize the Act access-latency penalty
  - ctxT [65, 2, sq] += v_aug.T @ pT accumulated over sk chunks in PSUM,
    then DMA'd to DRAM unnormalized; the host divides rows 0..63 by row 64
  - the projection matmul groups for later sq tiles are interleaved into the
    attention stream with depth-2 QK/exp lookahead, so the in-order PE queue
    never blocks on the Act engine
"""
import sys

if "/opt/trn_rl_repo" not in sys.path:
    sys.path.insert(0, "/opt/trn_rl_repo")

import numpy as np
import ml_dtypes

import concourse.bacc as bacc
import concourse.mybir as mybir
import concourse.tile as tile
from concourse.bass_utils import run_bass_kernel_spmd

F32 = mybir.dt.float32
BF16 = mybir.dt.bfloat16
F32R = mybir.dt.float32r

# matmul operand dtype: "bf16" or "f32r" (f32r self-loads weights, no
# separate Ldweights; <256-col matmuls run 4x slower so band trim differs)
DT_MODE = "bf16"

P = 128          # partitions / sk chunk
S = 2048         # sequence length
D = 1024         # model dim
C = 256          # W columns per core (4 heads x 64)
DH = 64          # head dim
NH = 4           # heads per core
SQT = 512        # sq tile (matmul free dim)
NSQ = S // SQT   # 4
NSK = S // P     # 16
ND = D // P      # 8
N_CORES = 8
N_WARM = 18      # warmup matmuls (~3us at mid clock)

_NC_CACHE = {}


def build_nc(loop_n=1, dt_mode=None):
    dt_mode = dt_mode or DT_MODE
    key = ("nc", loop_n, dt_mode)
    if key in _NC_CACHE:
        return _NC_CACHE[key]
    DT = BF16 if dt_mode == "bf16" else F32R
    MINW = 0 if dt_mode == "bf16" else 2 * P
    nc = bacc.Bacc("TRN2")
    xtq = nc.dram_tensor("xtq", [NSQ, P, ND, SQT], DT, kind="ExternalInput")
    wq = nc.dram_tensor("wq", [P, ND, C], DT, kind="ExternalInput")
    wk = nc.dram_tensor("wk", [P, ND, C], DT, kind="ExternalInput")
    wv = nc.dram_tensor("wv", [P, ND, C], DT, kind="ExternalInput")
    masks = nc.dram_tensor("masks", [P, 2, P], DT, kind="ExternalInput")
    octxT = nc.dram_tensor("octxT", [NSQ, 2, DH + 1, 2, SQT], F32,
                           kind="ExternalOutput")

    import contextlib
    with tile.TileContext(nc) as tc:
        with (tc.For_i(0, loop_n, 1) if loop_n > 1 else contextlib.nullcontext()), \
             tc.tile_pool(name="const", bufs=1) as cp, \
             tc.tile_pool(name="work", bufs=2) as wkp, \
             tc.tile_pool(name="ps", bufs=2, space="PSUM") as ps:
            # ---- persistent SBUF residents ----
            xt = cp.tile([P, ND, S], DT, tag="xt", name="xt")
            wq_sb = cp.tile([P, ND, C], DT, tag="wq", name="wq_sb")
            wk_sb = cp.tile([P, ND, C], DT, tag="wk", name="wk_sb")
            wv_sb = cp.tile([P, ND, C], DT, tag="wv", name="wv_sb")
            mask_sb = cp.tile([P, 2, P], DT, tag="mask", name="mask_sb")
            qT_sb = [cp.tile([P, S], DT, tag=f"qT{i}", name=f"qT{i}")
                     for i in range(2)]
            kT_sb = [cp.tile([P, S], DT, tag=f"kT{i}", name=f"kT{i}")
                     for i in range(2)]
            va = cp.tile([P, NSK, NH, DH + 1], DT, tag="va", name="va")
            warm_sb = cp.tile([P, C], DT, tag="warm", name="warm_sb")

            # ---- scratch init (DVE) ----
            nc.vector.memset(warm_sb[:], 0.0)
            nc.vector.memset(va[:, :, :, DH], 1.0)

            # ---- input DMAs ----
            # sync queue: x quarters (q0 split for earlier start)
            nc.sync.dma_start(xt[:, 0:4, 0:SQT], xtq[0, :, 0:4, :])
            nc.sync.dma_start(xt[:, 4:8, 0:SQT], xtq[0, :, 4:8, :])
            for q in range(1, NSQ):
                nc.sync.dma_start(xt[:, :, q * SQT:(q + 1) * SQT], xtq[q])
            # scalar queue (Act idle early): weights + mask
            nc.scalar.dma_start(wv_sb[:], wv[:])
            nc.scalar.dma_start(wq_sb[:], wq[:])
            nc.scalar.dma_start(wk_sb[:], wk[:])
            nc.scalar.dma_start(mask_sb[:], masks[:])

            # ---- warmup: keep PE busy + ramping while DMAs land ----
            for _ in range(N_WARM):
                s = ps.tile([P, 2, SQT], F32, tag="S", bufs=3, name="warm")
                nc.tensor.matmul(s[:, 0, 0:C], warm_sb[:, 0:P], warm_sb[:],
                                 start=True, stop=True)

            # ---- emit helpers ----
            def proj_v_mms(j, slot, ks):
                for k in ks:
                    nc.tensor.matmul(slot[:, 0, 0:C],
                                     xt[:, k, j * P:(j + 1) * P],
                                     wv_sb[:, k],
                                     start=(k == 0), stop=(k == ND - 1))

            def proj_v_copy(j, slot):
                nc.vector.tensor_copy(
                    va[:, j, :, 0:DH],
                    slot[:, 0, 0:C].rearrange("p (h d) -> p h d", h=NH))

            def proj_qk(w_sb, dst, hp, c):
                sq = slice(c * SQT, (c + 1) * SQT)
                slot = ps.tile([P, 2, SQT], F32, tag="S", bufs=3, name="pj")
                for k in range(ND):
                    nc.tensor.matmul(slot[:, 0],
                                     w_sb[:, k, hp * P:(hp + 1) * P],
                                     xt[:, k, sq],
                                     start=(k == 0), stop=(k == ND - 1))
                nc.vector.tensor_copy(dst[hp][:, sq], slot[:, 0])

            def v_filler(j, deadline=None):
                def f():
                    slot = ps.tile([P, 2, SQT], F32, tag="S", bufs=3,
                                   name="pv")
                    proj_v_mms(j, slot, range(ND))
                    proj_v_copy(j, slot)
                return (f, 5, deadline)

            def qk_filler(w_sb, dst, hp, c):
                def f():
                    proj_qk(w_sb, dst, hp, c)
                return (f, 10, None)

            def make_attn_groups(c, pscs_ref):
                """Attention groups for sq tile c: list of
                (emit_qk, emit_exp_mask, emit_pv) triples."""
                sq = slice(c * SQT, (c + 1) * SQT)
                jmax = 4 * c + 4
                groups = []

                def get_pscs(hp):
                    # one [65, 2, SQT] accumulator per (c, hp); bufs=1 so
                    # successive hp groups WAR-serialize on the output DMA
                    if pscs_ref[0] is None or pscs_ref[1] != hp:
                        pscs_ref[0] = ps.tile([DH + 1, 2, SQT], F32, tag="A",
                                              bufs=1, name="pscs")
                        pscs_ref[1] = hp
                    return pscs_ref[0]

                def u_group(hp, i, jp):
                    j0 = 2 * jp
                    off = DH * i
                    box = {}

                    def qk():
                        box["pscs"] = get_pscs(hp)
                        pss = ps.tile([P, 2, SQT], F32, tag="S", bufs=3,
                                      name="pss")
                        for u in range(2):
                            nc.tensor.matmul(
                                pss[:, u],
                                kT_sb[hp][off:off + DH,
                                          (j0 + u) * P:(j0 + u + 1) * P],
                                qT_sb[hp][off:off + DH, sq],
                                start=True, stop=True)
                        box["pss"] = pss

                    def ex():
                        pt = wkp.tile([P, 2, SQT], DT, tag="pT", bufs=8,
                                      name="pt")
                        nc.scalar.activation(pt[:], box["pss"][:],
                                             mybir.ActivationFunctionType.Exp,
                                             scale=0.125)
                        box["pt"] = pt

                    def pv():
                        h = 2 * hp + i
                        for u in range(2):
                            nc.tensor.matmul(box["pscs"][:, i],
                                             va[:, j0 + u, h, :],
                                             box["pt"][:, u],
                                             start=(j0 + u == 0), stop=False)

                    return (qk, ex, pv)

                def b_group(hp, t):
                    # both heads (i=0,1) of band tile t in one group
                    j = 4 * c + t
                    lo = min(P * t, SQT - MINW) if MINW else P * t
                    tri = P * t
                    w = SQT - lo
                    box = {}

                    def qk():
                        box["pscs"] = get_pscs(hp)
                        pss = ps.tile([P, 2, SQT], F32, tag="S", bufs=3,
                                      name="psb")
                        for i in range(2):
                            off = DH * i
                            nc.tensor.matmul(
                                pss[:, i, 0:w],
                                kT_sb[hp][off:off + DH, j * P:(j + 1) * P],
                                qT_sb[hp][off:off + DH,
                                          c * SQT + lo:(c + 1) * SQT],
                                start=True, stop=True)
                        box["pss"] = pss

                    def ex():
                        pt = wkp.tile([P, 2, SQT], DT, tag="pT", bufs=8,
                                      name="ptb")
                        nc.scalar.activation(pt[:, :, lo:SQT],
                                             box["pss"][:, :, 0:w],
                                             mybir.ActivationFunctionType.Exp,
                                             scale=0.125)
                        nc.vector.tensor_mul(pt[:, :, tri:tri + P],
                                             pt[:, :, tri:tri + P],
                                             mask_sb[:])
                        box["pt"] = pt

                    def pv():
                        pscs = box["pscs"]
                        for i in range(2):
                            nc.tensor.matmul(pscs[:, i, lo:SQT],
                                             va[:, j, 2 * hp + i, :],
                                             box["pt"][:, i, lo:SQT],
                                             start=(j == 0),
                                             stop=(j == jmax - 1))
                        if t == 3:
                            oc = wkp.tile([DH + 1, 2, SQT], F32, tag="oc",
                                          bufs=2, name="oc")
                            nc.vector.tensor_copy(oc[:], pscs[:])
                            nc.sync.dma_start(octxT[c, hp], oc[:])

                    return (qk, ex, pv)

                for hp in range(2):
                    for jp in range(2 * c):
                        for i in range(2):
                            groups.append(u_group(hp, i, jp))
                    for t in range(4):
                        groups.append(b_group(hp, t))
                return groups

            def run_pipeline(groups, fillers):
                # fillers: (fn, stride, deadline); schedule each at cumulative
                # stride positions clamped to its deadline (the last group
                # position before a consumer of its output is emitted),
                # leftovers after the last group
                n = len(groups)
                sched = {}
                pos = 0
                for idx, (fn, stride, deadline) in enumerate(fillers):
                    pos += stride
                    key = min(pos, n + idx + 1)
                    if deadline is not None:
                        key = min(key, deadline)
                        pos = min(pos, deadline)
                    sched.setdefault(key, []).append(fn)
                for g in range(min(2, n)):
                    groups[g][0]()
                    groups[g][1]()
                for g in range(n):
                    groups[g][2]()
                    for fn in sched.pop(g + 1, []):
                        fn()
                    if g + 2 < n:
                        groups[g + 2][0]()
                        groups[g + 2][1]()
                for key in sorted(sched):
                    for fn in sched[key]:
                        fn()

            # ---- prologue: projections for sq tile 0 (DMA-arrival aware) ----
            slots = {}
            for j in range(3):
                slots[j] = ps.tile([P, 2, SQT], F32, tag="S", bufs=3,
                                   name="pv0")
                proj_v_mms(j, slots[j], range(4))
            proj_v_mms(0, slots[0], range(4, ND))
            proj_v_copy(0, slots[0])
            slots[3] = ps.tile([P, 2, SQT], F32, tag="S", bufs=3, name="pv0")
            proj_v_mms(3, slots[3], range(4))
            for j in range(1, 4):
                proj_v_mms(j, slots[j], range(4, ND))
                proj_v_copy(j, slots[j])
            for hp in range(2):
                proj_qk(wq_sb, qT_sb, hp, 0)
                proj_qk(wk_sb, kT_sb, hp, 0)

            # ---- filler assignment ----
            #   A(0) <- all of proj(1); A(1) <- all of proj(2)
            #   A(2) <- qk of proj(3);  A(3) <- v of proj(3)
            def pv_fillers(c, same_stream=False):
                # same_stream: fillers run inside A(c) itself, so chunk
                # 4c+t must be emitted before band group 4c+t+1 (hp=0)
                return [v_filler(j, deadline=(j if same_stream else None))
                        for j in range(4 * c, 4 * c + 4)]

            def pqk_fillers(c):
                out = []
                for hp in range(2):
                    out.append(qk_filler(wq_sb, qT_sb, hp, c))
                    out.append(qk_filler(wk_sb, kT_sb, hp, c))
                return out

            filler_map = {
                0: pv_fillers(1) + pqk_fillers(1),
                1: pv_fillers(2) + pqk_fillers(2),
                2: pqk_fillers(3),
                3: pv_fillers(3, same_stream=True),
            }

            for c in range(NSQ):
                pscs_ref = [None, None]
                groups = make_attn_groups(c, pscs_ref)
                run_pipeline(groups, filler_map[c])
    nc.compile()
    _NC_CACHE[key] = nc
    return nc


def make_in_maps(x, Wq, Wk, Wv, dt_mode=None):
    dt_mode = dt_mode or DT_MODE
    bf = ml_dtypes.bfloat16 if dt_mode == "bf16" else np.float32
    x = np.asarray(x, dtype=np.float32)
    Wq = np.asarray(Wq, dtype=np.float32)
    Wk = np.asarray(Wk, dtype=np.float32)
    Wv = np.asarray(Wv, dtype=np.float32)
    tri = (np.arange(P)[:, None] <= np.arange(P)[None, :])
    masks = np.ascontiguousarray(
        np.broadcast_to(tri[:, None, :], (P, 2, P)).astype(bf))
    in_maps = []
    for core in range(N_CORES):
        b, g = divmod(core, 4)
        cols = slice(C * g, C * (g + 1))
        xtq = np.ascontiguousarray(
            x[b].T.reshape(ND, P, NSQ, SQT).transpose(2, 1, 0, 3).astype(bf))
        in_maps.append({
            "xtq": xtq,
            "wq": np.ascontiguousarray(
                Wq[:, cols].reshape(ND, P, C).transpose(1, 0, 2).astype(bf)),
            "wk": np.ascontiguousarray(
                Wk[:, cols].reshape(ND, P, C).transpose(1, 0, 2).astype(bf)),
            "wv": np.ascontiguousarray(
                Wv[:, cols].reshape(ND, P, C).transpose(1, 0, 2).astype(bf)),
            "masks": masks,
        })
    return in_maps


def assemble_out(results):
    out = np.empty((2, S, D), np.float32)
    for core in range(N_CORES):
        b, g = divmod(core, 4)
        oc = results[core]["octxT"]               # [NSQ, 2, DH+1, 2, SQT]
        ctx = oc[:, :, 0:DH] / oc[:, :, DH:DH + 1]   # [NSQ, 2, DH, 2, SQT]
        # [q, hp, d, i, f] -> [q, f, hp, i, d] -> [S, C]
        out[b, :, C * g:C * (g + 1)] = (
            ctx.transpose(0, 4, 1, 3, 2).reshape(S, C))
    return out


def kernel(x, Wq, Wk, Wv):
    nc = build_nc()
    in_maps = make_in_maps(x, Wq, Wk, Wv)
    res = run_bass_kernel_spmd(nc, in_maps, core_ids=list(range(N_CORES)))
    return assemble_out(res.results)
